# revision 1
# baseline (speedup 1.0000x reference)
"""Sparse attention (RoPE'd Q=K, strictly-causal unnormalized scores @ V).

  Q: (1, 4, 2048, 8192) f32   V: (1, 1, 2048, 256) f32
  out = tril(QR @ QR^T, -1) @ V   per head, V broadcast over heads.

Sharding: 8 cores = 4 heads x 2 halves of the N=8192 contraction dim.
The causal mask is elementwise, so masked-scores @ V is additive over
N-slices: each core computes a full (2048, 256) partial output from its
(2048, 4096) slice of QR; host sums the two halves per head.

Device algorithm (chunked linear attention, chunk C=256):
  out[t] = QR[t] @ S_{<chunk} + (intra-chunk causal part), where
  S = sum_s QR[s] (x) V[s] is an [N_c, D] state accumulated chunk by chunk.
  MACs: 2*T*N_c*D + T*C*N_c ~= 6.3e9/core vs 1.7e10 for direct causal.

All matmuls run as float32r (TF32-class precision, 1 cycle/row PE rate at
free dim >= 256). Inputs are host-packed so each chunk is a single large
DMA with 32 KiB contiguous descriptor runs:
  qrt_p[c]  = [128, 32*256]  (SBUF layout: partition p=n%128, k-tile major)
  qtn_p[c]  = [128, 2*4096]  (partition p=t%128 within chunk)
  v_p       = [128, 16*256]
Host does RoPE; f32 bytes are declared float32r on device.
"""

import math

import numpy as np

THETA = 2.0**16
TWO_PI = 2.0 * math.pi

B, NH, T, N, D = 1, 4, 2048, 8192, 256
NSPLIT = 2
NCORES = NH * NSPLIT
NC_FEAT = N // NSPLIT  # 4096 features per core
P = 128
KT = NC_FEAT // P  # 32 n-tiles
TT = T // P  # 16 t-tiles
C = 256  # chunk length
NCH = T // C  # 8 chunks
CSUB = C // P  # 2 t-subtiles per chunk

_COMPILED = None
_ROPE_E = None


def _rope_tables():
    """cos/sin as one complex table; frequencies are pair-constant, so only
    even columns are needed. Input-independent -> cached across calls."""
    global _ROPE_E
    if _ROPE_E is None:
        idx = (np.floor(np.arange(N, dtype=np.float32) / 2.0) * 2.0).astype(
            np.float32
        )
        freqs = (1.0 / (THETA ** (idx / np.float32(N))) / np.float32(TWO_PI)).astype(
            np.float32
        )
        t = np.arange(T, dtype=np.float32)
        phases = t[:, None] * freqs[None, ::2]
        ang = np.float32(TWO_PI) * (phases % np.float32(1.0))
        E = np.empty((T, N // 2), np.complex64)
        E.real = np.cos(ang)
        E.imag = np.sin(ang)
        _ROPE_E = E
    return _ROPE_E


def _rope_host(Q):
    """(a+bi)(c+si) = (ac-bs) + (as+bc)i == the reference's interleaved
    rotate-pairs RoPE, one pass over Q viewed as complex64."""
    E = _rope_tables()
    QRc = Q.view(np.complex64) * E
    return QRc.view(np.float32)


def _masks_host():
    """mask[i][si, tj] = 1 if si + 128*i < tj, variants i=0,1 over C cols."""
    si = np.arange(P)[:, None]
    tj = np.arange(C)[None, :]
    return np.concatenate(
        [(si + P * i < tj).astype(np.float32) for i in range(CSUB)], axis=0
    )  # [256, 256]


def _build():
    import concourse.tile as tile
    from concourse import bacc, mybir

    nc = bacc.Bacc(
        "TRN2",
        target_bir_lowering=False,
        debug=False,
        enable_asserts=False,
        num_devices=NCORES,
    )
    f32 = mybir.dt.float32
    f32r = mybir.dt.float32r

    qrt = nc.dram_tensor("qrt", [NCH, P, KT * C], f32r, kind="ExternalInput").ap()
    qtn = nc.dram_tensor(
        "qtn", [NCH, P, CSUB * NC_FEAT], f32r, kind="ExternalInput"
    ).ap()
    v = nc.dram_tensor("v", [P, TT * D], f32r, kind="ExternalInput").ap()
    masks = nc.dram_tensor("masks", [CSUB * P, C], f32, kind="ExternalInput").ap()
    out = nc.dram_tensor("out", [T, D], f32, kind="ExternalOutput").ap()

    with tile.TileContext(nc) as tc:
        with (
            tc.tile_pool(name="qr", bufs=5) as qp,
            tc.tile_pool(name="qt", bufs=3) as tp,
            tc.tile_pool(name="vp", bufs=1) as vp,
            tc.tile_pool(name="mk", bufs=CSUB) as mp,
            tc.tile_pool(name="st", bufs=KT) as stp,
            tc.tile_pool(name="sc", bufs=4) as sp,
            tc.tile_pool(name="ob", bufs=3) as op_,
            tc.tile_pool(name="pi", bufs=2, space="PSUM") as ppi,
            tc.tile_pool(name="po", bufs=2, space="PSUM") as ppo,
            tc.tile_pool(name="pu", bufs=3, space="PSUM") as ppu,
        ):
            vtiles = None
            mtiles = None
            Stiles = [
                stp.tile([P, D], f32r, tag="S", name=f"S{k}") for k in range(KT)
            ]

            for c in range(NCH):
                c0 = c * C
                qh = []
                for u in range(2):
                    qhu = qp.tile(
                        [P, KT * C // 2], f32r, tag="qr", name=f"q{c}_{u}"
                    )
                    nc.sync.dma_start(
                        out=qhu,
                        in_=qrt[c][:, u * (KT * C // 2) : (u + 1) * (KT * C // 2)],
                    )
                    qh.append(qhu)

                def qslice(k, lo, hi):
                    u, kk = divmod(k, KT // 2)
                    return qh[u][:, kk * C + lo : kk * C + hi]

                if c == 0:
                    # emitted after chunk 0's qrt so the scheduler streams
                    # that first (intra_0 only needs qrt and masks)
                    mtiles = []
                    for i in range(CSUB):
                        mt = mp.tile([P, C], f32)
                        nc.sync.dma_start(out=mt, in_=masks[i * P : (i + 1) * P, :])
                        mtiles.append(mt)
                    vt = vp.tile([P, TT * D], f32r)
                    nc.sync.dma_start(out=vt, in_=v)
                    vtiles = [vt[:, a * D : (a + 1) * D] for a in range(TT)]

                tn_half = []
                if c < NCH - 1:
                    for m in range(CSUB):
                        tnh = tp.tile(
                            [P, NC_FEAT], f32r, tag="tn", name=f"tn{c}_{m}"
                        )
                        nc.sync.dma_start(
                            out=tnh,
                            in_=qtn[c][:, m * NC_FEAT : (m + 1) * NC_FEAT],
                        )
                        tn_half.append(tnh)

                # intra-chunk causal scores, [s, t] upper layout
                st_c = []
                for a in range(CSUB):
                    ps = ppi.tile([P, C], f32)
                    for k in range(KT):
                        nc.tensor.matmul(
                            ps,
                            lhsT=qslice(k, a * P, a * P + P),
                            rhs=qslice(k, 0, C),
                            start=(k == 0),
                            stop=(k == KT - 1),
                        )
                    st = sp.tile([P, C], f32r)
                    nc.vector.tensor_mul(st, ps, mtiles[a])
                    st_c.append(st)

                # out rows of this chunk: q @ S_{<c} + intra @ V
                ot = op_.tile([P, CSUB * D], f32)
                for m in range(CSUB):
                    po = ppo.tile([P, D], f32)
                    first = True
                    if c > 0:
                        for k in range(KT):
                            nc.tensor.matmul(
                                po,
                                lhsT=qslice(k, m * P, m * P + P),
                                rhs=Stiles[k],
                                start=first,
                                stop=False,
                            )
                            first = False
                    for a in range(m + 1):
                        nc.tensor.matmul(
                            po,
                            lhsT=st_c[a][:, m * P : (m + 1) * P],
                            rhs=vtiles[CSUB * c + a],
                            start=first,
                            stop=(a == m),
                        )
                        first = False
                    nc.vector.tensor_copy(ot[:, m * D : (m + 1) * D], po)
                out_rows = out[c0 : c0 + C, :].rearrange("(m p) d -> p m d", p=P)
                nc.sync.dma_start(
                    out=out_rows, in_=ot.rearrange("p (m d) -> p m d", m=CSUB)
                )

                # state update: S[k] += qtn_c[:, k-tile].T @ V_chunk
                # (the state after the last chunk is never read)
                if c == NCH - 1:
                    continue
                for k in range(KT):
                    pu = ppu.tile([P, D], f32)
                    for m in range(CSUB):
                        nc.tensor.matmul(
                            pu,
                            lhsT=tn_half[m][:, k * P : k * P + P],
                            rhs=vtiles[CSUB * c + m],
                            start=(m == 0),
                            stop=(m == CSUB - 1),
                        )
                    if c == 0:
                        nc.vector.tensor_copy(Stiles[k], pu)
                    else:
                        nc.vector.tensor_add(Stiles[k], Stiles[k], pu)

    nc.compile()
    return nc


def _get_compiled():
    global _COMPILED
    if _COMPILED is None:
        _COMPILED = _build()
    return _COMPILED


def _pack_core(sl):
    """sl: [T, NC_FEAT] f32 QR slice for one core -> (qrt_p, qtn_p)."""
    # qrt_p[c, p, k*C+j] = sl[c*C+j, k*128+p]
    qrt_p = np.ascontiguousarray(
        sl.reshape(NCH, C, KT, P).transpose(0, 3, 2, 1).reshape(NCH, P, KT * C)
    )
    # qtn_p[c, p, m*NC_FEAT+n] = sl[c*C+m*128+p, n]
    qtn_p = np.ascontiguousarray(
        sl.reshape(NCH, CSUB, P, NC_FEAT).transpose(0, 2, 1, 3).reshape(
            NCH, P, CSUB * NC_FEAT
        )
    )
    return qrt_p, qtn_p


def kernel(Q, V, _want_results=False, **_unused):
    from concourse import bass_utils

    Q = np.asarray(Q, dtype=np.float32)
    V = np.asarray(V, dtype=np.float32)

    QR = _rope_host(Q)  # (1, 4, 2048, 8192) f32
    masks_np = _masks_host()
    # v_p[p, a*D+d] = V[0, 0, a*128+p, d]
    v_p = np.ascontiguousarray(
        V[0, 0].reshape(TT, P, D).transpose(1, 0, 2).reshape(P, TT * D)
    )

    in_maps = []
    for h in range(NH):
        for half in range(NSPLIT):
            sl = QR[0, h, :, half * NC_FEAT : (half + 1) * NC_FEAT]
            qrt_p, qtn_p = _pack_core(sl)
            in_maps.append(
                {"qrt": qrt_p, "qtn": qtn_p, "v": v_p, "masks": masks_np}
            )

    nc = _get_compiled()
    res = bass_utils.run_bass_kernel_spmd(nc, in_maps, core_ids=list(range(NCORES)))

    out = np.empty((B, NH, T, D), dtype=np.float32)
    for h in range(NH):
        out[0, h] = res.results[2 * h]["out"] + res.results[2 * h + 1]["out"]
    if _want_results:
        return out, res
    return out


if __name__ == "__main__":
    rng = np.random.default_rng(0)
    Q = (rng.standard_normal((B, NH, T, N)) * 0.02).astype(np.float32)
    V = rng.standard_normal((B, 1, T, D)).astype(np.float32)
    out = kernel(Q=Q, V=V)
    print("out", out.shape, out.dtype, float(np.abs(out).max()))

